# revision 9
# baseline (speedup 1.0000x reference)
"""NeighborConsistencyLoss on 8 Trainium2 NeuronCores.

Math:  loss = mean_s(1 - mean_k cos(z[s], z[knn[s,k]]))
            = 1 - (1/(S*K)) * sum_{s,k} u(z[s]) . u(z[knn[s,k]])
where u(x) = x/|x| (eps in max(|a||b|, eps) never binds for randn data).

Sharding: replicate z (host-cast to bf16: halves gather bytes and feeds
the PE directly), shard the S=1000 sampled centers across 8 cores.

v7 design, driven by HW traces of earlier versions:
 - Gathers use classic indirect1d (InstDMACopy, one row-offset per
   partition, 128 rows per instruction): its SWDGE descriptor
   generation is ~0.9ns/desc vs ~7ns/desc for InstDMAGatherAnt, and it
   avoids the ~8us one-time Q7 library swap dma_gather triggers.
 - The 33 neighbor-tile gathers + 1 center gather are spread across
   the 4 SWDGE queues (num_swdge_queues=4, inst.queue =
   "qPoolDynamic{q}"): each queue has an async desc-gen worker, so
   four gathers generate descriptors concurrently (~10us wall instead
   of ~46us serial). A 64KB descriptor ring (dynamic_dma_scratch_size)
   keeps the workers from stalling on ring space.
 - Structured tile layout (no index banking, no pads, no host masks):
   idx[p, t] is the row for tile t partition p (tile t covers centers
   4t..4t+3, partition p -> neighbor p%32 of center 4t+p//32); col 33
   holds center rows.
 - Per tile: ssq via ACT Square+accum (3 of 5) / DVE mult+accum (2 of
   5), rnorm = 1/sqrt batched per 4 tiles, mask_rn = wmask_slice *
   rnorm in fp8 (DVE, [128,128] op), then the PE folds neighbor-sum
   AND normalization into one accumulating matmul:
   V[s,:] += mask_rn^T @ raw_bf16_rows  (lhsT fp8, rhs bf16).
 - wmask is the constant 0/1 sliding block mask from v1, stored fp8.
 - Center: norm early; r[s] = rno_c[s] * sum_d c[s,d]*V[s,d] (fused
   DVE op), partial = cmask^T @ r, host sums partials:
   loss = 1 - total/(S*K).
"""

import numpy as np

N, D, K, S = 200000, 512, 32, 1000
NCORES = 8
SPC = S // NCORES            # 125 samples per core
P = 128
NT = 32                      # neighbor tiles per core (4 centers x 32 rows)
NTT = NT + 1                 # +1 odd tile for centers 128..? no: col NT = centers
GRP = 4

_cache = {}


def _build_module():
    import concourse.bacc as bacc
    import concourse.bass as bass
    import concourse.mybir as mybir
    import concourse.tile as tile

    f32 = mybir.dt.float32
    bf16 = mybir.dt.bfloat16
    fp8 = mybir.dt.float8e4
    i32 = mybir.dt.int32
    AF = mybir.ActivationFunctionType
    ALU = mybir.AluOpType

    nc = bacc.Bacc(None, target_bir_lowering=False,
                   dynamic_dma_scratch_size=2**16, num_swdge_queues=4)
    z_t = nc.dram_tensor("z", [N, D], bf16, kind="ExternalInput")
    idx_t = nc.dram_tensor("idx", [P, NT + 1], i32, kind="ExternalInput")
    w_t = nc.dram_tensor("wmask", [P, 256], fp8, kind="ExternalInput")
    cmask_t = nc.dram_tensor("cmask", [P, 1], f32, kind="ExternalInput")
    out_t = nc.dram_tensor("out", [1, 1], f32, kind="ExternalOutput")

    def gather(out_ap, off_ap, q):
        bi = nc.gpsimd.indirect_dma_start(
            out=out_ap, out_offset=None, in_=z_t[:],
            in_offset=bass.IndirectOffsetOnAxis(ap=off_ap, axis=0),
        )
        if q:
            bi.ins.queue = f"qPoolDynamic{q}"
        return bi

    with tile.TileContext(nc) as tc:
        with (
            tc.tile_pool(name="const", bufs=1) as const,
            tc.tile_pool(name="gath", bufs=1) as gath,
            tc.tile_pool(name="scr", bufs=2) as scr,
            tc.tile_pool(name="mrn", bufs=4) as mrn,
            tc.tile_pool(name="ps", bufs=1, space="PSUM") as ps,
        ):
            idx_sb = const.tile([P, NT + 1], i32, tag="idx")
            nc.sync.dma_start(idx_sb[:], idx_t[:])
            w_sb = const.tile([P, 256], fp8, tag="wmask")
            nc.sync.dma_start(w_sb[:], w_t[:])
            cmask_sb = const.tile([P, 1], f32, tag="cmask")
            nc.sync.dma_start(cmask_sb[:], cmask_t[:])

            # warm both activation tables during the dead startup window
            warm = const.tile([P, 1], f32, tag="warm")
            warm2 = const.tile([P, 1], f32, tag="warm2")
            nc.vector.memset(warm[:], 1.0)
            nc.scalar.activation(warm2[:], warm[:], AF.Square)
            nc.scalar.activation(warm2[:], warm[:], AF.Sqrt)

            # center rows first (queue 3), then neighbor tiles round-robin
            ctile = gath.tile([P, D], bf16, tag="ctile")
            gather(ctile[:], idx_sb[:, NT:NT + 1], 3)

            chunks = []
            for t in range(NT):
                ch = gath.tile([P, D], bf16, tag=f"ch{t}")
                gather(ch[:], idx_sb[:, t:t + 1], t % 4)
                chunks.append(ch)

            # center norm (early)
            ssq_c = const.tile([P, 1], f32, tag="ssqc")
            sqr_c = const.tile([P, 1], f32, tag="sqrc")
            rno_c = const.tile([P, 1], f32, tag="rnoc")
            sc0 = scr.tile([P, D], bf16, tag="dve_sq")
            nc.vector.scalar_tensor_tensor(
                out=sc0[:], in0=ctile[:], scalar=1.0, in1=ctile[:],
                op0=ALU.mult, op1=ALU.mult, accum_out=ssq_c[:],
            )
            nc.scalar.activation(sqr_c[:], ssq_c[:], AF.Sqrt)
            nc.vector.reciprocal(rno_c[:], sqr_c[:])

            V = ps.tile([P, D], f32, tag="V")
            ssq = const.tile([P, NT], f32, tag="ssq")
            sqr = const.tile([P, NT], f32, tag="sqr")
            rno = const.tile([P, NT], f32, tag="rno")

            for g0 in range(0, NT, GRP):
                g1 = min(g0 + GRP, NT)
                for t in range(g0, g1):
                    src = chunks[t][:]
                    if t % 5 < 3:
                        sc = scr.tile([P, D], bf16, tag="act_sq")
                        nc.scalar.activation(
                            sc[:], src, AF.Square, accum_out=ssq[:, t:t + 1]
                        )
                    else:
                        sc = scr.tile([P, D], bf16, tag="dve_sq")
                        nc.vector.scalar_tensor_tensor(
                            out=sc[:], in0=src, scalar=1.0, in1=src,
                            op0=ALU.mult, op1=ALU.mult,
                            accum_out=ssq[:, t:t + 1],
                        )
                nc.scalar.activation(sqr[:, g0:g1], ssq[:, g0:g1], AF.Sqrt)
                nc.vector.reciprocal(rno[:, g0:g1], sqr[:, g0:g1])
                for t in range(g0, g1):
                    m = mrn.tile([P, P], fp8, tag="mrn")
                    nc.vector.tensor_scalar_mul(
                        m[:], w_sb[:, 124 - 4 * t:252 - 4 * t], rno[:, t:t + 1]
                    )
                    nc.tensor.matmul(
                        out=V[:], lhsT=m[:], rhs=chunks[t][:],
                        start=(t == 0), stop=(t == NT - 1),
                    )

            wscr = scr.tile([P, D], f32, tag="wscr")
            r = const.tile([P, 1], f32, tag="r")
            nc.vector.scalar_tensor_tensor(
                out=wscr[:], in0=ctile[:], scalar=rno_c[:, :1], in1=V[:],
                op0=ALU.mult, op1=ALU.mult, accum_out=r[:],
            )

            res_ps = ps.tile([1, 1], f32, tag="res")
            nc.tensor.matmul(
                out=res_ps[:], lhsT=cmask_sb[:], rhs=r[:], start=True, stop=True
            )
            res_sb = const.tile([1, 1], f32, tag="res_sb")
            nc.vector.tensor_copy(res_sb[:], res_ps[:])
            nc.sync.dma_start(out_t[:], res_sb[:])

    nc.compile()
    return nc


def _get_module():
    if "nc" not in _cache:
        _cache["nc"] = _build_module()
    return _cache["nc"]


def _make_in_maps(z, knn_neighbors, sample_indices):
    import ml_dtypes

    z = np.asarray(z, dtype=np.float32)
    knn = np.asarray(knn_neighbors).astype(np.int64)
    sample = np.asarray(sample_indices).astype(np.int64).ravel()
    assert z.shape == (N, D) and knn.shape == (N, K) and sample.shape == (S,)

    z_bf = np.ascontiguousarray(z.astype(ml_dtypes.bfloat16))

    # sliding-window block mask: w[p, c] = 1 iff c == 124 + p//32, so the
    # [128,128] slice at col offset 124-4t gives the lhsT mask for tile t
    pp = np.arange(P)
    w = np.zeros((P, 256), dtype=ml_dtypes.float8_e4m3)
    w[pp, 124 + pp // 32] = 1.0
    maskv = (pp < SPC).astype(np.float32).reshape(P, 1)

    in_maps = []
    for c in range(NCORES):
        s_ids = np.zeros(P, dtype=np.int64)
        s_ids[:SPC] = sample[c * SPC:(c + 1) * SPC]
        nb = knn[s_ids]                               # [128, 32]
        idx = np.empty((P, NT + 1), dtype=np.int32)
        for t in range(NT):
            idx[:, t] = nb[4 * t + pp // 32, pp % 32]
        idx[:, NT] = s_ids
        in_maps.append({"z": z_bf, "idx": idx, "wmask": w, "cmask": maskv})
    return in_maps


def _combine(results):
    total = sum(float(res["out"][0, 0]) for res in results)
    return np.array(1.0 - total / (S * K), dtype=np.float32)


def kernel(z, knn_neighbors, sample_indices):
    from concourse.bass_utils import run_bass_kernel_spmd

    nc = _get_module()
    in_maps = _make_in_maps(z, knn_neighbors, sample_indices)
    out = run_bass_kernel_spmd(nc, in_maps, core_ids=list(range(NCORES)))
    return _combine(out.results)


def run_profiled(z, knn_neighbors, sample_indices, **kw):
    """Dev helper: same as kernel() but returns (loss, BassKernelResults)
    with trace/profile enabled."""
    from concourse.bass_utils import run_bass_kernel_spmd

    nc = _get_module()
    in_maps = _make_in_maps(z, knn_neighbors, sample_indices)
    out = run_bass_kernel_spmd(
        nc, in_maps, core_ids=list(range(NCORES)), trace=True, **kw
    )
    return _combine(out.results), out
